# revision 17
# baseline (speedup 1.0000x reference)
"""Trainium2 Bass kernel for nn_BasicBlock (FBS-masked ternary conv + BN + LeakyReLU).

Sharding: data-parallel over batch. B=32 -> 4 samples per core on 8 cores.
BN batch stats are AllGathered per cot and reduced locally.

v3 design (vs v2 baseline at 330.7us):
  - Conv in fp8e4m3 with perf_mode=DoubleRow (0.5 cyc/row vs bf16's 1.0).
    x is split into (hi, lo) fp8 planes (hi = fp8(x), lo = fp8(x - hi)) that
    ride the DoubleRow pair dimension with a stride-0 stationary AP, so the
    fp8 quantization of x self-cancels (~0.1% residual).  The ternary weight
    values {0, 1, r8} are exactly representable in fp8 (r8 = fp8-round of
    neg/pos; its 0.4% rounding error lands on the BN-normalized output).
  - No quadrant deinterleave: conv matmuls read strided (stride-2) rhs APs
    straight from the dense [ci, 2(lvl), 64, 64] fp8 sample tiles.
  - t = 0.05*max|W| and r8 are scalar immediates (host-derived like r_imm
    was in v2), removing the weight-max reduction from the critical path.
  - Emission is hand-interleaved so each engine's in-order queue matches
    wall-clock order (v2 queued all |x| accums before the first weight
    transpose eviction, serializing the prologue).
  - Conv unit order (cot,b): all cots on b0 first, then per-cot remaining
    samples so each cot's AllGather fires as early as possible; only the
    last cot's AllGather (15us) is exposed in the tail.
  - Epilogue is single-pass Act Prelu (scale/shift/leaky in one op).
"""

import numpy as np

import concourse.bass as bass
import concourse.mybir as mybir
import concourse.tile as tile
from concourse.bass_utils import run_bass_kernel_spmd
from concourse.masks import make_identity

F32 = mybir.dt.float32
F8 = mybir.dt.float8e4
BF16 = mybir.dt.bfloat16
AF = mybir.ActivationFunctionType
ALU = mybir.AluOpType
AX = mybir.AxisListType
PM = mybir.MatmulPerfMode

N_CORES = 8
B, CIN, H, W = 32, 256, 64, 64
COUT, KK = 512, 4
OH, OW = 32, 32
NB = B // N_CORES          # samples per core = 4
NT = CIN // 128            # ci tiles = 2
NCOT = COUT // 128         # co tiles = 4
CR_KEEP = 409.5            # count <= 409  <->  count < 409.5
BN_EPS = 1e-5
NEG_SLOPE = 0.2
THRESH_FACTOR = 0.05
NSP = OH * OW              # 1024 spatial positions per sample
BIG = 1.0e30

MAX_WAITS = 1              # this walrus build allows 1 sync wait per instruction

# kh -> (row parity ph, row shift dj): x row 2*oh + kh - 1 = 2*(oh+dj) + ph
PAR = {0: (1, -1), 1: (0, 0), 2: (1, 0), 3: (0, 1)}
KHW_ORDER = ([(1, 1)]
             + [(kh, kw) for kh in range(KK) for kw in range(KK)
                if (kh, kw) != (1, 1) and kh * KK + kw < 8]
             + [(kh, kw) for kh in range(KK) for kw in range(KK)
                if kh * KK + kw >= 8])

# conv unit order: all cots see sample 0 early (one ramp per cot), then each
# cot finishes its remaining samples in turn so AllGathers fire spread out.
UNIT_ORDER = [(0, 0), (1, 0), (2, 0), (3, 0),
              (0, 1), (0, 2), (1, 1), (1, 2), (0, 3), (1, 3),
              (2, 1), (2, 2), (2, 3), (3, 1), (3, 2), (3, 3)]


def _split_waits(nc, max_waits=MAX_WAITS):
    """Split per-instruction sem waits exceeding max_waits into preceding
    same-engine InstNoOp carriers (engines execute their queue in order)."""
    for f in nc.m.functions:
        for bb in f.blocks:
            new_list = []
            changed = False
            for ins in bb.instructions:
                si = ins.sync_info
                if si is not None and si.on_wait and len(si.on_wait) > max_waits:
                    waits = list(si.on_wait)
                    carry = waits[: len(waits) - max_waits]
                    keep = waits[len(waits) - max_waits:]
                    k = 0
                    while carry:
                        chunk, carry = carry[:max_waits], carry[max_waits:]
                        new_list.append(
                            mybir.InstNoOp(
                                name=f"{ins.name}_ws{k}",
                                engine=ins.engine,
                                bass_nofuse=True,
                                sync_info=mybir.SyncInfo(on_wait=chunk, on_update=[]),
                            )
                        )
                        k += 1
                    ins.sync_info = mybir.SyncInfo(
                        on_wait=keep, on_update=list(si.on_update)
                    )
                    changed = True
                new_list.append(ins)
            if changed:
                bb.instructions = new_list


def _fp8_round(v: float) -> float:
    """Nearest fp8e4m3 value (normal range; |v| in [2^-6, 448])."""
    import math
    if v == 0.0:
        return 0.0
    s = math.copysign(1.0, v)
    m, e = math.frexp(abs(v))          # |v| = m * 2^e, m in [0.5, 1)
    k = round(m * 16)
    if k == 16:
        k, e = 8, e + 1
    return s * (k / 16.0) * (2.0 ** e)


def build_kernel(r8: float, eps_imm: float, t_imm: float, debug: bool = False):
    """Per-core module. r8 = fp8(neg/pos), eps_imm = eps/pos^2, t = 0.05*max|W|."""
    nc = bass.Bass()

    xs = nc.dram_tensor("xs", [NB, CIN, H, W], F32, kind="ExternalInput")
    wt = nc.dram_tensor("wt", [COUT, CIN, KK, KK], F32, kind="ExternalInput")
    salw = nc.dram_tensor("salw", [COUT, CIN], F32, kind="ExternalInput")
    salb = nc.dram_tensor("salb", [COUT], F32, kind="ExternalInput")
    gam = nc.dram_tensor("gam", [COUT], F32, kind="ExternalInput")
    bet = nc.dram_tensor("bet", [COUT], F32, kind="ExternalInput")
    out = nc.dram_tensor("out", [NB, COUT, OH, OW], F32, kind="ExternalOutput")

    cc_out = nc.dram_tensor("cc_out", [NCOT, N_CORES, 2, 128], F32,
                            addr_space="Shared")
    if debug:
        dbg_sub = nc.dram_tensor("dbg_sub", [128, NT * NB], F32, kind="ExternalOutput")
        dbg_sal = nc.dram_tensor("dbg_sal", [128, NCOT * NB], F32, kind="ExternalOutput")
        dbg_mask = nc.dram_tensor("dbg_mask", [128, NCOT * NB], F32, kind="ExternalOutput")
        dbg_thr = nc.dram_tensor("dbg_thr", [NB], F32, kind="ExternalOutput")
        dbg_wq = nc.dram_tensor("dbg_wq", [128, 32 * 128], F8, kind="ExternalOutput")
        dbg_x8 = nc.dram_tensor("dbg_x8", [128, 2, 2, H, W], F8, kind="ExternalOutput")
        dbg_scl = nc.dram_tensor("dbg_scl", [128, NCOT], F32, kind="ExternalOutput")
        dbg_shf = nc.dram_tensor("dbg_shf", [128, NCOT], F32, kind="ExternalOutput")

    with tile.TileContext(nc) as tc:
        with (
            tc.tile_pool(name="persist", bufs=1) as pp,
            tc.tile_pool(name="big", bufs=2) as bigp,
            tc.tile_pool(name="wq", bufs=4) as wqp,
            tc.tile_pool(name="stage", bufs=3) as stp,
            tc.tile_pool(name="small", bufs=2) as smp,
            tc.tile_pool(name="ps", bufs=8, space="PSUM") as psp,
            tc.tile_pool(name="dram", bufs=1, space="DRAM") as dp,
        ):
            # ---------- constants ----------
            identF = pp.tile([128, 128], F32, name="identF")
            make_identity(nc, identF)
            identB = pp.tile([128, 128], BF16, name="identB")
            make_identity(nc, identB)
            onesP1 = pp.tile([1, 128], F32, name="onesP1")
            nc.vector.memset(onesP1, 1.0)
            ones1 = pp.tile([128, 1], BF16, name="ones1")
            nc.vector.memset(ones1, 1.0)
            epst = pp.tile([128, 1], F32, name="epst")
            nc.vector.memset(epst, float(eps_imm))
            tcol = pp.tile([128, 1], F32, name="tcol")
            nc.vector.memset(tcol, float(t_imm))
            ntcol = pp.tile([128, 1], F32, name="ntcol")
            nc.vector.memset(ntcol, float(-t_imm))

            # ---------- persistent tiles ----------
            # x8[b]: [ci' 128, t(2), lvl(2), 64, 64] fp8; lvl0 = fp8(x), lvl1 = residual
            x8 = [bigp.tile([128, NT, 2, H, W], F8, name=f"x8_{b}",
                            tag="x8", bufs=3) for b in range(NB)]
            subT = [pp.tile([128, NB], F32, name=f"subT{t}") for t in range(NT)]
            ascr = pp.tile([128, H * W], F8, name="ascr")
            salwT = [pp.tile([128, COUT], F32, name=f"swT{t}") for t in range(NT)]
            sal_cb = [pp.tile([128, NB], F32, name=f"salcb{c}") for c in range(NCOT)]
            mask_cb = pp.tile([128, NCOT * NB], F32, name="mask_cb")
            thrB = [pp.tile([128, 1], F32, name=f"thrB{b}") for b in range(NB)]
            yv = [pp.tile([128, NB * NSP], BF16, name=f"yv{c}") for c in range(NCOT)]
            s1 = [pp.tile([128, 2 * NB], F32, name=f"s1_{c}") for c in range(NCOT)]
            s2 = [pp.tile([128, 2 * NB], F32, name=f"s2_{c}") for c in range(NCOT)]
            scl = pp.tile([128, NCOT], F32, name="scl")
            shf = pp.tile([128, NCOT], F32, name="shf")
            wq = [wqp.tile([128, 32 * 128], F8, name=f"wq{c}", tag="wq")
                  for c in range(NCOT)]
            thr_d = dp.tile([NB, 1], F32, name="thr_d")
            cc_in = dp.tile([NCOT, 128, 2], F32, name="cc_in")
            inv_hw = 1.0 / float(H * W)
            inv_n = 1.0 / float(B * NSP)

            # ---------- emission helpers ----------
            wch = {}

            def emit_wch_dma(c):
                wc = bigp.tile([128, NT * 128 * 16], F32, name=f"wch{c}", tag="big")
                nc.sync.dma_start(
                    out=wc,
                    in_=wt[c * 128:(c + 1) * 128, :, :, :].rearrange(
                        "co ci kh kw -> co (ci kh kw)"))
                wch[c] = wc

            wcht = {}

            def emit_tern(c, h, eng):
                # wcht[c][co, (khw t ci')] = [w>t] + r8*[w<-t]  (bf16; exact)
                wc = wch[c]
                if c not in wcht:
                    wcht[c] = smp.tile([128, NT * 128 * 16], BF16,
                                       name=f"wcht{c}", tag="wcht", bufs=2)
                wh = wcht[c]
                src_h = bass.AP(
                    tensor=wc.tensor, offset=wc.offset + h * 8,
                    ap=[wc.ap[0], [1, 8], [128 * 16, NT], [16, 128]])
                dst_h = bass.AP(
                    tensor=wh.tensor, offset=wh.offset + h * 8 * NT * 128,
                    ap=[wh.ap[0], [NT * 128, 8], [128, NT], [1, 128]])
                eng.tensor_scalar(dst_h, src_h, ntcol[:, :], float(r8),
                                  op0=ALU.is_lt, op1=ALU.mult)
                eng.scalar_tensor_tensor(
                    out=dst_h, in0=src_h, scalar=tcol[:, 0:1], in1=dst_h,
                    op0=ALU.is_gt, op1=ALU.add)

            def emit_T(c, gbs):
                # transpose groups gb (4 [128,128] transposes per PSUM bank)
                for gb in gbs:
                    pbk = psp.tile([128, 512], BF16, name=f"tw{c}{gb}", tag="aux", bufs=3)
                    for k in range(4):
                        g = gb * 4 + k
                        nc.tensor.transpose(
                            pbk[:, k * 128:(k + 1) * 128],
                            wcht[c][:, g * 128:(g + 1) * 128], identB)
                    nc.scalar.activation(
                        wq[c][:, gb * 512:(gb + 1) * 512], pbk, AF.Copy)

            def emit_x_block(b, t):
                stg = stp.tile([128, H, W], F32, name=f"x{b}{t}", tag="stage",
                               bufs=2)
                nc.sync.dma_start(out=stg, in_=xs[b, t * 128:(t + 1) * 128, :, :])
                nc.scalar.activation(
                    ascr, stg.rearrange("p a b -> p (a b)"), AF.Abs,
                    accum_out=subT[t][:, b:b + 1])
                hi = x8[b][:, t, 0, :, :]
                nc.gpsimd.tensor_copy(out=hi, in_=stg)
                nc.vector.tensor_tensor(
                    out=x8[b][:, t, 1, :, :], in0=stg, in1=hi, op=ALU.subtract)

            def emit_unit(c, b):
                banks = [psp.tile([128, 512], F32, name=f"bk{c}{b}{n}", tag="bank",
                                  bufs=5) for n in range(2)]
                bank_cnt = [0, 0]
                for t in range(NT):
                    for kh, kw in KHW_ORDER:
                        ph, dj = PAR[kh]
                        pw, di = PAR[kw]
                        cl = max(0, -di)
                        ch_ = min(OW - 1, OW - 1 - di)
                        g = (kh * KK + kw) * NT + t
                        lhsT = bass.AP(
                            tensor=wq[c].tensor, offset=wq[c].offset + g * 128,
                            ap=[wq[c].ap[0], [0, 2], [1, 128]])
                        for n in range(2):
                            oh_lo = max(16 * n, -dj)
                            oh_hi = min(16 * n + 15, OH - 1 - dj)
                            n_oh = oh_hi - oh_lo + 1
                            n_ow = ch_ - cl + 1
                            row0 = 2 * (oh_lo + dj) + ph
                            col0 = 2 * (cl + di) + pw
                            rhs = bass.AP(
                                tensor=x8[b].tensor,
                                offset=(x8[b].offset + t * 2 * H * W
                                        + row0 * W + col0),
                                ap=[x8[b].ap[0], [H * W, 2],
                                    [2 * W, n_oh], [2, n_ow]])
                            dst = banks[n].rearrange(
                                "p (r cc) -> p r cc", r=16)[
                                :, oh_lo - 16 * n: oh_hi - 16 * n + 1,
                                cl: ch_ + 1]
                            nc.tensor.matmul(
                                dst, lhsT, rhs,
                                start=(bank_cnt[n] == 0),
                                stop=(bank_cnt[n] == NT * 16 - 1),
                                perf_mode=PM.DoubleRow,
                                skip_group_check=True)
                            bank_cnt[n] += 1
                return banks

            def topk_sample(b):
                # exact per-sample top-k threshold + mask (channel-partition)
                psal = psp.tile([128, 512], F32, name=f"psal{b}", tag="aux", bufs=3)
                for c in range(NCOT):
                    for t in range(NT):
                        nc.tensor.matmul(psal[:, c:c + 1],
                                         salwT[t][:, c * 128:(c + 1) * 128],
                                         subT[t][:, b:b + 1],
                                         start=(t == 0), stop=(t == NT - 1))
                    nc.scalar.activation(sal_cb[c][:, b:b + 1], psal[:, c:c + 1],
                                         AF.Abs, bias=salb_t[:, c:c + 1],
                                         scale=inv_hw)
                prow = psp.tile([128, 512], F32, name=f"prow{b}", tag="aux", bufs=3)
                for c in range(NCOT):
                    nc.tensor.transpose(prow[0:1, c * 128:(c + 1) * 128],
                                        sal_cb[c][:, b:b + 1], identF)
                salrow = smp.tile([1, COUT], F32, name=f"srow{b}", tag="srow", bufs=1)
                nc.vector.tensor_copy(out=salrow, in_=prow[0:1, :])
                pbc = psp.tile([128, 512], F32, name=f"pbc{b}", tag="aux", bufs=3)
                nc.tensor.matmul(pbc, onesP1, salrow, start=True, stop=True)
                # count[j] = #{p : sal_j < sal_p} via compare + ones-matmul
                pcnt = psp.tile([128, 512], F32, name=f"pcnt{b}", tag="aux", bufs=3)
                for c in range(NCOT):
                    cmp = smp.tile([128, COUT], BF16, name=f"cmp{b}{c}",
                                   tag="cmp", bufs=2)
                    nc.vector.tensor_scalar(cmp, pbc, sal_cb[c][:, b:b + 1],
                                            None, op0=ALU.is_lt)
                    nc.tensor.matmul(pcnt[0:1, :], ones1, cmp,
                                     start=(c == 0), stop=(c == NCOT - 1))
                t3 = smp.tile([1, COUT], F32, name=f"t3{b}", tag="t3", bufs=1)
                nc.vector.tensor_scalar(t3, pcnt[0:1, :], CR_KEEP, BIG,
                                        op0=ALU.is_ge, op1=ALU.mult)
                nc.vector.tensor_tensor(out=t3, in0=t3, in1=pbc[0:1, :],
                                        op=ALU.max)
                thr = smp.tile([1, 1], F32, name=f"thr{b}", tag="thr")
                nc.vector.tensor_reduce(thr, t3, axis=AX.X, op=ALU.min)
                nc.scalar.dma_start(out=thr_d[b, :], in_=thr)
                nc.scalar.dma_start(
                    out=thrB[b],
                    in_=bass.AP(tensor=thr_d.tensor, offset=thr_d.offset + b,
                                ap=[[0, 128], [1, 1]]))
                for c in range(NCOT):
                    nc.vector.scalar_tensor_tensor(
                        out=mask_cb[:, c * NB + b: c * NB + b + 1],
                        in0=sal_cb[c][:, b:b + 1], scalar=thrB[b][:, 0:1],
                        in1=sal_cb[c][:, b:b + 1], op0=ALU.is_gt, op1=ALU.mult)

            def emit_evict(c, b, banks):
                for n in range(2):
                    slot = b * 2 + n
                    ysl = yv[c][:, b * NSP + n * 512: b * NSP + (n + 1) * 512]
                    nc.scalar.activation(
                        ysl, banks[n], AF.Copy, bias=0.0,
                        scale=mask_cb[:, c * NB + b: c * NB + b + 1],
                        accum_out=s1[c][:, slot:slot + 1])
                    sq = stp.tile([128, 512], BF16, name=f"sq{c}{b}{n}",
                                  tag="sq", bufs=2)
                    nc.vector.scalar_tensor_tensor(
                        out=sq, in0=banks[n],
                        scalar=mask_cb[:, c * NB + b: c * NB + b + 1],
                        in1=ysl, op0=ALU.mult, op1=ALU.mult,
                        accum_out=s2[c][:, slot:slot + 1])

            def emit_ag_front(c):
                r12 = smp.tile([128, 2], F32, name=f"r12_{c}", tag="r12", bufs=4)
                nc.vector.tensor_reduce(r12[:, 0:1], s1[c], axis=AX.X, op=ALU.add)
                nc.vector.tensor_reduce(r12[:, 1:2], s2[c], axis=AX.X, op=ALU.add)
                nc.gpsimd.dma_start(out=cc_in[c, :, :], in_=r12)
                nc.gpsimd.collective_compute(
                    "AllGather", ALU.bypass,
                    replica_groups=[list(range(N_CORES))],
                    ins=[cc_in[c, :, :]], outs=[cc_out[c, :, :, :]])

            def emit_ag_back(c):
                sg = smp.tile([128, 2 * N_CORES], F32, name=f"sg{c}", tag="sg")
                nc.scalar.dma_start(
                    out=sg, in_=bass.AP(tensor=cc_out, offset=c * 2 * 128 * N_CORES,
                                        ap=[[2, 128], [256, N_CORES], [1, 2]]))
                s12 = smp.tile([128, 2], F32, name=f"s12_{c}", tag="s12")
                sgv = bass.AP(tensor=sg.tensor, offset=sg.offset,
                              ap=[sg.ap[0], [1, 2], [2, N_CORES]])
                nc.vector.tensor_reduce(s12, sgv, axis=AX.X, op=ALU.add)
                mu = smp.tile([128, 1], F32, name=f"mu{c}", tag="mu", bufs=1)
                nc.vector.tensor_scalar(mu, s12[:, 0:1], inv_n, None, op0=ALU.mult)
                m2 = smp.tile([128, 1], F32, name=f"m2{c}", tag="m2", bufs=1)
                nc.vector.tensor_scalar(m2, s12[:, 1:2], inv_n, None, op0=ALU.mult)
                var = smp.tile([128, 1], F32, name=f"var{c}", tag="var", bufs=1)
                nc.vector.scalar_tensor_tensor(
                    out=var, in0=mu, scalar=mu[:, :], in1=m2,
                    op0=ALU.mult, op1=ALU.subtract)  # mu*mu - m2 = -var
                sv = smp.tile([128, 1], F32, name=f"sv{c}", tag="sv", bufs=1)
                nc.scalar.activation(sv, var, AF.Sqrt, bias=epst[:, :], scale=-1.0)
                rstd = smp.tile([128, 1], F32, name=f"rstd{c}", tag="rstd", bufs=1)
                nc.vector.reciprocal(rstd, sv)
                nc.vector.tensor_tensor(out=scl[:, c:c + 1],
                                        in0=gam_t[:, c:c + 1], in1=rstd,
                                        op=ALU.mult)
                msc = smp.tile([128, 1], F32, name=f"msc{c}", tag="msc", bufs=1)
                nc.vector.tensor_tensor(out=msc, in0=mu,
                                        in1=scl[:, c:c + 1], op=ALU.mult)
                nc.vector.tensor_tensor(out=shf[:, c:c + 1],
                                        in0=bet_t[:, c:c + 1], in1=msc,
                                        op=ALU.subtract)

            def emit_epi(c):
                for b in range(NB):
                    z = stp.tile([128, NSP], F32, name=f"z{c}{b}", tag="z", bufs=2)
                    for n in range(2):
                        ysl = yv[c][:, b * NSP + n * 512: b * NSP + (n + 1) * 512]
                        zn = z[:, n * 512:(n + 1) * 512]
                        if c == NCOT - 1 and n == 1:
                            nc.vector.tensor_scalar(zn, ysl, scl[:, c:c + 1],
                                                    shf[:, c:c + 1],
                                                    op0=ALU.mult, op1=ALU.add)
                            nc.vector.scalar_tensor_tensor(
                                out=zn, in0=zn, scalar=float(NEG_SLOPE),
                                in1=zn, op0=ALU.mult, op1=ALU.max)
                        else:
                            nc.scalar.activation(
                                zn, ysl, AF.Prelu, bias=shf[:, c:c + 1],
                                scale=scl[:, c:c + 1], alpha=float(NEG_SLOPE))
                    nc.sync.dma_start(
                        out=out[b, c * 128:(c + 1) * 128, :, :].rearrange(
                            "p h w -> p (h w)"),
                        in_=z)

            # ================= emission sequence =================
            emit_wch_dma(0)
            emit_tern(0, 0, nc.vector)
            emit_tern(0, 1, nc.vector)
            emit_T(0, range(8))
            emit_x_block(0, 0)
            emit_wch_dma(1)

            # saliency weights + per-channel vectors
            swn = []
            for c in range(NCOT):
                sw = smp.tile([128, CIN], F32, name=f"swn{c}", tag="salw")
                nc.sync.dma_start(out=sw, in_=salw[c * 128:(c + 1) * 128, :])
                swn.append(sw)

            def col128(dram_vec, nm):  # [512] dram -> [128,4] sbuf
                t_ = pp.tile([128, NCOT], F32, name=nm)
                ap = bass.AP(tensor=dram_vec, offset=0, ap=[[1, 128], [128, NCOT]])
                nc.sync.dma_start(out=t_, in_=ap)
                return t_

            salb_t = col128(salb, "salb_t")
            gam_t = col128(gam, "gam_t")
            bet_t = col128(bet, "bet_t")

            for c in range(NCOT):
                for t in range(NT):
                    pbt = psp.tile([128, 512], F32, name=f"ptw{c}{t}", tag="aux", bufs=3)
                    nc.tensor.transpose(pbt[:, 0:128],
                                        swn[c][:, t * 128:(t + 1) * 128], identF)
                    nc.scalar.copy(salwT[t][:, c * 128:(c + 1) * 128],
                                   pbt[:, 0:128])

            emit_x_block(0, 1)
            emit_tern(1, 0, nc.vector)
            emit_tern(1, 1, nc.vector)

            seq_banks = {}
            seq_banks[(0, 0)] = emit_unit(0, 0)
            topk_sample(0)
            emit_evict(0, 0, seq_banks.pop((0, 0)))

            emit_wch_dma(2)
            emit_x_block(1, 0)
            emit_tern(2, 0, nc.vector)
            emit_tern(2, 1, nc.vector)
            emit_T(1, range(8))

            seq_banks[(1, 0)] = emit_unit(1, 0)
            emit_evict(1, 0, seq_banks.pop((1, 0)))

            emit_wch_dma(3)
            emit_x_block(1, 1)
            topk_sample(1)
            emit_tern(3, 0, nc.vector)
            emit_tern(3, 1, nc.vector)
            emit_T(2, range(8))

            seq_banks[(2, 0)] = emit_unit(2, 0)
            emit_evict(2, 0, seq_banks.pop((2, 0)))

            emit_x_block(2, 0)
            emit_T(3, range(8))

            seq_banks[(3, 0)] = emit_unit(3, 0)
            emit_evict(3, 0, seq_banks.pop((3, 0)))

            emit_x_block(2, 1)
            topk_sample(2)

            banks = emit_unit(0, 1)
            emit_evict(0, 1, banks)

            emit_x_block(3, 0)

            banks = emit_unit(0, 2)
            emit_evict(0, 2, banks)

            emit_x_block(3, 1)
            topk_sample(3)

            banks = emit_unit(1, 1)
            emit_evict(1, 1, banks)

            banks = emit_unit(0, 3)
            emit_evict(0, 3, banks)
            emit_ag_front(0)

            banks = emit_unit(1, 2)
            emit_evict(1, 2, banks)

            banks = emit_unit(1, 3)
            emit_evict(1, 3, banks)
            emit_ag_front(1)

            for (c, b) in [(2, 1), (2, 2), (2, 3)]:
                banks = emit_unit(c, b)
                emit_evict(c, b, banks)
            emit_ag_front(2)

            for b in (1, 2, 3):
                banks = emit_unit(3, b)
                emit_evict(3, b, banks)
            emit_ag_front(3)

            # finale: stats finalize + epilogue per cot (emitted last so the
            # sg waits never block conv-phase work on any queue)
            for c in range(NCOT):
                emit_ag_back(c)
                emit_epi(c)

            if debug:
                nc.sync.dma_start(out=dbg_mask[:, :], in_=mask_cb)
                for t in range(NT):
                    nc.sync.dma_start(out=bass.AP(tensor=dbg_sub, offset=t * NB,
                                                  ap=[[NT * NB, 128], [1, NB]]),
                                      in_=subT[t])
                for c in range(NCOT):
                    nc.sync.dma_start(out=bass.AP(tensor=dbg_sal, offset=c * NB,
                                                  ap=[[NCOT * NB, 128], [1, NB]]),
                                      in_=sal_cb[c])
                nc.sync.dma_start(out=dbg_thr[:],
                                  in_=bass.AP(tensor=thr_d.tensor,
                                              offset=thr_d.offset,
                                              ap=[[1, NB]]))
                nc.sync.dma_start(out=dbg_wq[:, :], in_=wq[3])
                nc.sync.dma_start(out=dbg_x8[:, :, :, :, :], in_=x8[0])
                nc.sync.dma_start(out=dbg_scl[:, :], in_=scl)
                nc.sync.dma_start(out=dbg_shf[:, :], in_=shf)

    _split_waits(nc)
    return nc


_CACHE = {}


def kernel(x, weight, pos, neg, sal_w, sal_b, gamma, beta):
    x = np.ascontiguousarray(np.asarray(x, dtype=np.float32))
    weight = np.ascontiguousarray(np.asarray(weight, dtype=np.float32))
    sal_w = np.ascontiguousarray(np.asarray(sal_w, dtype=np.float32))
    sal_b = np.ascontiguousarray(np.asarray(sal_b, dtype=np.float32))
    gamma = np.ascontiguousarray(np.asarray(gamma, dtype=np.float32))
    beta = np.ascontiguousarray(np.asarray(beta, dtype=np.float32))
    pos_f = np.float32(np.asarray(pos).reshape(()))
    neg_f = np.float32(np.asarray(neg).reshape(()))

    r8 = _fp8_round(float(np.float32(neg_f / pos_f)))
    eps_imm = float(np.float32(BN_EPS) / (pos_f * pos_f))
    t_imm = float(np.float32(THRESH_FACTOR) * np.float32(np.abs(weight).max()))

    import os
    debug = os.environ.get("KERNEL_DEBUG", "0") == "1"
    key = (r8, eps_imm, t_imm, debug)
    if key not in _CACHE:
        _CACHE[key] = build_kernel(r8, eps_imm, t_imm, debug)
    nc = _CACHE[key]

    in_maps = []
    for c in range(N_CORES):
        in_maps.append({
            "xs": x[c * NB:(c + 1) * NB],
            "wt": weight,
            "salw": sal_w,
            "salb": sal_b,
            "gam": gamma,
            "bet": beta,
        })
    res = run_bass_kernel_spmd(nc, in_maps, core_ids=list(range(N_CORES)))
    if debug:
        kernel.dbg = res.results
    out = np.concatenate([res.results[c]["out"] for c in range(N_CORES)], axis=0)
    return out


# revision 18
# speedup vs baseline: 1.0270x; 1.0270x over previous
"""Trainium2 Bass kernel for nn_BasicBlock (FBS-masked ternary conv + BN + LeakyReLU).

Sharding: data-parallel over batch. B=32 -> 4 samples per core on 8 cores.
BN batch stats are AllGathered per cot and reduced locally.

v3 design (vs v2 baseline at 330.7us):
  - Conv in fp8e4m3 with perf_mode=DoubleRow (0.5 cyc/row vs bf16's 1.0).
    x is split into (hi, lo) fp8 planes (hi = fp8(x), lo = fp8(x - hi)) that
    ride the DoubleRow pair dimension with a stride-0 stationary AP, so the
    fp8 quantization of x self-cancels (~0.1% residual).  The ternary weight
    values {0, 1, r8} are exactly representable in fp8 (r8 = fp8-round of
    neg/pos; its 0.4% rounding error lands on the BN-normalized output).
  - No quadrant deinterleave: conv matmuls read strided (stride-2) rhs APs
    straight from the dense [ci, 2(lvl), 64, 64] fp8 sample tiles.
  - t = 0.05*max|W| and r8 are scalar immediates (host-derived like r_imm
    was in v2), removing the weight-max reduction from the critical path.
  - Emission is hand-interleaved so each engine's in-order queue matches
    wall-clock order (v2 queued all |x| accums before the first weight
    transpose eviction, serializing the prologue).
  - Conv unit order (cot,b): all cots on b0 first, then per-cot remaining
    samples so each cot's AllGather fires as early as possible; only the
    last cot's AllGather (15us) is exposed in the tail.
  - Epilogue is single-pass Act Prelu (scale/shift/leaky in one op).
"""

import numpy as np

import concourse.bass as bass
import concourse.mybir as mybir
import concourse.tile as tile
from concourse.bass_utils import run_bass_kernel_spmd
from concourse.masks import make_identity

F32 = mybir.dt.float32
F8 = mybir.dt.float8e4
BF16 = mybir.dt.bfloat16
AF = mybir.ActivationFunctionType
ALU = mybir.AluOpType
AX = mybir.AxisListType
PM = mybir.MatmulPerfMode

N_CORES = 8
B, CIN, H, W = 32, 256, 64, 64
COUT, KK = 512, 4
OH, OW = 32, 32
NB = B // N_CORES          # samples per core = 4
NT = CIN // 128            # ci tiles = 2
NCOT = COUT // 128         # co tiles = 4
CR_KEEP = 409.5            # count <= 409  <->  count < 409.5
BN_EPS = 1e-5
NEG_SLOPE = 0.2
THRESH_FACTOR = 0.05
NSP = OH * OW              # 1024 spatial positions per sample
BIG = 1.0e30

MAX_WAITS = 1              # this walrus build allows 1 sync wait per instruction

# kh -> (row parity ph, row shift dj): x row 2*oh + kh - 1 = 2*(oh+dj) + ph
PAR = {0: (1, -1), 1: (0, 0), 2: (1, 0), 3: (0, 1)}
KHW_ORDER = ([(1, 1)]
             + [(kh, kw) for kh in range(KK) for kw in range(KK)
                if (kh, kw) != (1, 1) and kh * KK + kw < 8]
             + [(kh, kw) for kh in range(KK) for kw in range(KK)
                if kh * KK + kw >= 8])

# conv unit order: all cots see sample 0 early (one ramp per cot), then each
# cot finishes its remaining samples in turn so AllGathers fire spread out.
UNIT_ORDER = [(0, 0), (1, 0), (2, 0), (3, 0),
              (0, 1), (0, 2), (1, 1), (1, 2), (0, 3), (1, 3),
              (2, 1), (2, 2), (2, 3), (3, 1), (3, 2), (3, 3)]


def _split_waits(nc, max_waits=MAX_WAITS):
    """Split per-instruction sem waits exceeding max_waits into preceding
    same-engine InstNoOp carriers (engines execute their queue in order)."""
    for f in nc.m.functions:
        for bb in f.blocks:
            new_list = []
            changed = False
            for ins in bb.instructions:
                si = ins.sync_info
                if si is not None and si.on_wait and len(si.on_wait) > max_waits:
                    waits = list(si.on_wait)
                    carry = waits[: len(waits) - max_waits]
                    keep = waits[len(waits) - max_waits:]
                    k = 0
                    while carry:
                        chunk, carry = carry[:max_waits], carry[max_waits:]
                        new_list.append(
                            mybir.InstNoOp(
                                name=f"{ins.name}_ws{k}",
                                engine=ins.engine,
                                bass_nofuse=True,
                                sync_info=mybir.SyncInfo(on_wait=chunk, on_update=[]),
                            )
                        )
                        k += 1
                    ins.sync_info = mybir.SyncInfo(
                        on_wait=keep, on_update=list(si.on_update)
                    )
                    changed = True
                new_list.append(ins)
            if changed:
                bb.instructions = new_list


def _fp8_round(v: float) -> float:
    """Nearest fp8e4m3 value (normal range; |v| in [2^-6, 448])."""
    import math
    if v == 0.0:
        return 0.0
    s = math.copysign(1.0, v)
    m, e = math.frexp(abs(v))          # |v| = m * 2^e, m in [0.5, 1)
    k = round(m * 16)
    if k == 16:
        k, e = 8, e + 1
    return s * (k / 16.0) * (2.0 ** e)


def build_kernel(r8: float, eps_imm: float, t_imm: float, debug: bool = False):
    """Per-core module. r8 = fp8(neg/pos), eps_imm = eps/pos^2, t = 0.05*max|W|."""
    nc = bass.Bass()

    xs = nc.dram_tensor("xs", [NB, CIN, H, W], F32, kind="ExternalInput")
    wt = nc.dram_tensor("wt", [COUT, CIN, KK, KK], F32, kind="ExternalInput")
    salw = nc.dram_tensor("salw", [COUT, CIN], F32, kind="ExternalInput")
    salb = nc.dram_tensor("salb", [COUT], F32, kind="ExternalInput")
    gam = nc.dram_tensor("gam", [COUT], F32, kind="ExternalInput")
    bet = nc.dram_tensor("bet", [COUT], F32, kind="ExternalInput")
    out = nc.dram_tensor("out", [NB, COUT, OH, OW], F32, kind="ExternalOutput")

    cc_out = nc.dram_tensor("cc_out", [NCOT, N_CORES, 2, 128], F32,
                            addr_space="Shared")
    if debug:
        dbg_sub = nc.dram_tensor("dbg_sub", [128, NT * NB], F32, kind="ExternalOutput")
        dbg_sal = nc.dram_tensor("dbg_sal", [128, NCOT * NB], F32, kind="ExternalOutput")
        dbg_mask = nc.dram_tensor("dbg_mask", [128, NCOT * NB], F32, kind="ExternalOutput")
        dbg_thr = nc.dram_tensor("dbg_thr", [NB], F32, kind="ExternalOutput")
        dbg_wq = nc.dram_tensor("dbg_wq", [128, 32 * 128], F8, kind="ExternalOutput")
        dbg_x8 = nc.dram_tensor("dbg_x8", [128, 2, 2, H, W], F8, kind="ExternalOutput")
        dbg_scl = nc.dram_tensor("dbg_scl", [128, NCOT], F32, kind="ExternalOutput")
        dbg_shf = nc.dram_tensor("dbg_shf", [128, NCOT], F32, kind="ExternalOutput")

    with tile.TileContext(nc) as tc:
        with (
            tc.tile_pool(name="persist", bufs=1) as pp,
            tc.tile_pool(name="big", bufs=2) as bigp,
            tc.tile_pool(name="wq", bufs=4) as wqp,
            tc.tile_pool(name="stage", bufs=3) as stp,
            tc.tile_pool(name="small", bufs=2) as smp,
            tc.tile_pool(name="ps", bufs=8, space="PSUM") as psp,
            tc.tile_pool(name="dram", bufs=1, space="DRAM") as dp,
        ):
            # ---------- constants ----------
            identF = pp.tile([128, 128], F32, name="identF")
            make_identity(nc, identF)
            identB = pp.tile([128, 128], BF16, name="identB")
            make_identity(nc, identB)
            onesP1 = pp.tile([1, 128], F32, name="onesP1")
            nc.vector.memset(onesP1, 1.0)
            ones1 = pp.tile([128, 1], BF16, name="ones1")
            nc.vector.memset(ones1, 1.0)
            epst = pp.tile([128, 1], F32, name="epst")
            nc.vector.memset(epst, float(eps_imm))
            tcol = pp.tile([128, 1], F32, name="tcol")
            nc.vector.memset(tcol, float(t_imm))
            ntcol = pp.tile([128, 1], F32, name="ntcol")
            nc.vector.memset(ntcol, float(-t_imm))

            # ---------- persistent tiles ----------
            # x8[b]: [ci' 128, t(2), lvl(2), 64, 64] fp8; lvl0 = fp8(x), lvl1 = residual
            x8 = [bigp.tile([128, NT, 2, H, W], F8, name=f"x8_{b}",
                            tag="x8", bufs=3) for b in range(NB)]
            subT = [pp.tile([128, NB], F32, name=f"subT{t}") for t in range(NT)]
            ascr = pp.tile([128, H * W], F8, name="ascr")
            salwT = [pp.tile([128, COUT], F32, name=f"swT{t}") for t in range(NT)]
            sal_cb = [pp.tile([128, NB], F32, name=f"salcb{c}") for c in range(NCOT)]
            mask_cb = pp.tile([128, NCOT * NB], F32, name="mask_cb")
            thrB = [pp.tile([128, 1], F32, name=f"thrB{b}") for b in range(NB)]
            yv = [pp.tile([128, NB * NSP], BF16, name=f"yv{c}") for c in range(NCOT)]
            s1 = [pp.tile([128, 2 * NB], F32, name=f"s1_{c}") for c in range(NCOT)]
            s2 = [pp.tile([128, 2 * NB], F32, name=f"s2_{c}") for c in range(NCOT)]
            scl = pp.tile([128, NCOT], F32, name="scl")
            shf = pp.tile([128, NCOT], F32, name="shf")
            wq = [wqp.tile([128, 32 * 128], F8, name=f"wq{c}", tag="wq")
                  for c in range(NCOT)]
            thr_d = dp.tile([NB, 1], F32, name="thr_d")
            cc_in = dp.tile([NCOT, 128, 2], F32, name="cc_in")
            inv_hw = 1.0 / float(H * W)
            inv_n = 1.0 / float(B * NSP)

            # ---------- emission helpers ----------
            wch = {}

            def emit_wch_dma(c):
                wc = bigp.tile([128, NT * 128 * 16], F32, name=f"wch{c}", tag="big")
                nc.sync.dma_start(
                    out=wc,
                    in_=wt[c * 128:(c + 1) * 128, :, :, :].rearrange(
                        "co ci kh kw -> co (ci kh kw)"))
                wch[c] = wc

            wcht = {}

            def emit_tern(c, h, eng):
                # wcht[c][co, (khw t ci')] = [w>t] + r8*[w<-t]  (bf16; exact)
                wc = wch[c]
                if c not in wcht:
                    wcht[c] = smp.tile([128, NT * 128 * 16], BF16,
                                       name=f"wcht{c}", tag="wcht", bufs=2)
                wh = wcht[c]
                src_h = bass.AP(
                    tensor=wc.tensor, offset=wc.offset + h * 8,
                    ap=[wc.ap[0], [1, 8], [128 * 16, NT], [16, 128]])
                dst_h = bass.AP(
                    tensor=wh.tensor, offset=wh.offset + h * 8 * NT * 128,
                    ap=[wh.ap[0], [NT * 128, 8], [128, NT], [1, 128]])
                eng.tensor_scalar(dst_h, src_h, ntcol[:, :], float(r8),
                                  op0=ALU.is_lt, op1=ALU.mult)
                eng.scalar_tensor_tensor(
                    out=dst_h, in0=src_h, scalar=tcol[:, 0:1], in1=dst_h,
                    op0=ALU.is_gt, op1=ALU.add)

            def emit_T(c, gbs):
                # transpose groups gb (4 [128,128] transposes per PSUM bank)
                for gb in gbs:
                    pbk = psp.tile([128, 512], BF16, name=f"tw{c}{gb}", tag="wbank", bufs=2)
                    for k in range(4):
                        g = gb * 4 + k
                        nc.tensor.transpose(
                            pbk[:, k * 128:(k + 1) * 128],
                            wcht[c][:, g * 128:(g + 1) * 128], identB)
                    nc.scalar.activation(
                        wq[c][:, gb * 512:(gb + 1) * 512], pbk, AF.Copy)

            def emit_x_block(b, t):
                stg = stp.tile([128, H, W], F32, name=f"x{b}{t}", tag="stage",
                               bufs=2)
                nc.sync.dma_start(out=stg, in_=xs[b, t * 128:(t + 1) * 128, :, :])
                nc.scalar.activation(
                    ascr, stg.rearrange("p a b -> p (a b)"), AF.Abs,
                    accum_out=subT[t][:, b:b + 1])
                hi = x8[b][:, t, 0, :, :]
                nc.gpsimd.tensor_copy(out=hi, in_=stg)
                nc.vector.tensor_tensor(
                    out=x8[b][:, t, 1, :, :], in0=stg, in1=hi, op=ALU.subtract)

            def emit_unit(c, b):
                banks = [psp.tile([128, 512], F32, name=f"bk{c}{b}{n}", tag="bank",
                                  bufs=4) for n in range(2)]
                bank_cnt = [0, 0]
                for t in range(NT):
                    for kh, kw in KHW_ORDER:
                        ph, dj = PAR[kh]
                        pw, di = PAR[kw]
                        cl = max(0, -di)
                        ch_ = min(OW - 1, OW - 1 - di)
                        g = (kh * KK + kw) * NT + t
                        lhsT = bass.AP(
                            tensor=wq[c].tensor, offset=wq[c].offset + g * 128,
                            ap=[wq[c].ap[0], [0, 2], [1, 128]])
                        for n in range(2):
                            oh_lo = max(16 * n, -dj)
                            oh_hi = min(16 * n + 15, OH - 1 - dj)
                            n_oh = oh_hi - oh_lo + 1
                            n_ow = ch_ - cl + 1
                            row0 = 2 * (oh_lo + dj) + ph
                            col0 = 2 * (cl + di) + pw
                            rhs = bass.AP(
                                tensor=x8[b].tensor,
                                offset=(x8[b].offset + t * 2 * H * W
                                        + row0 * W + col0),
                                ap=[x8[b].ap[0], [H * W, 2],
                                    [2 * W, n_oh], [2, n_ow]])
                            dst = banks[n].rearrange(
                                "p (r cc) -> p r cc", r=16)[
                                :, oh_lo - 16 * n: oh_hi - 16 * n + 1,
                                cl: ch_ + 1]
                            nc.tensor.matmul(
                                dst, lhsT, rhs,
                                start=(bank_cnt[n] == 0),
                                stop=(bank_cnt[n] == NT * 16 - 1),
                                perf_mode=PM.DoubleRow,
                                skip_group_check=True)
                            bank_cnt[n] += 1
                return banks

            def topk_sample(b):
                # exact per-sample top-k threshold + mask (channel-partition)
                psal = psp.tile([128, 512], F32, name=f"psal{b}", tag="tbank", bufs=2)
                for c in range(NCOT):
                    for t in range(NT):
                        nc.tensor.matmul(psal[:, c:c + 1],
                                         salwT[t][:, c * 128:(c + 1) * 128],
                                         subT[t][:, b:b + 1],
                                         start=(t == 0), stop=(t == NT - 1))
                    nc.scalar.activation(sal_cb[c][:, b:b + 1], psal[:, c:c + 1],
                                         AF.Abs, bias=salb_t[:, c:c + 1],
                                         scale=inv_hw)
                prow = psp.tile([128, 512], F32, name=f"prow{b}", tag="tbank", bufs=2)
                for c in range(NCOT):
                    nc.tensor.transpose(prow[0:1, c * 128:(c + 1) * 128],
                                        sal_cb[c][:, b:b + 1], identF)
                salrow = smp.tile([1, COUT], F32, name=f"srow{b}", tag="srow", bufs=1)
                nc.vector.tensor_copy(out=salrow, in_=prow[0:1, :])
                pbc = psp.tile([128, 512], F32, name=f"pbc{b}", tag="tbank", bufs=2)
                nc.tensor.matmul(pbc, onesP1, salrow, start=True, stop=True)
                # count[j] = #{p : sal_j < sal_p} via compare + ones-matmul
                pcnt = psp.tile([128, 512], F32, name=f"pcnt{b}", tag="tbank", bufs=2)
                for c in range(NCOT):
                    cmp = smp.tile([128, COUT], BF16, name=f"cmp{b}{c}",
                                   tag="cmp", bufs=2)
                    nc.vector.tensor_scalar(cmp, pbc, sal_cb[c][:, b:b + 1],
                                            None, op0=ALU.is_lt)
                    nc.tensor.matmul(pcnt[0:1, :], ones1, cmp,
                                     start=(c == 0), stop=(c == NCOT - 1))
                t3 = smp.tile([1, COUT], F32, name=f"t3{b}", tag="t3", bufs=1)
                nc.vector.tensor_scalar(t3, pcnt[0:1, :], CR_KEEP, BIG,
                                        op0=ALU.is_ge, op1=ALU.mult)
                nc.vector.tensor_tensor(out=t3, in0=t3, in1=pbc[0:1, :],
                                        op=ALU.max)
                thr = smp.tile([1, 1], F32, name=f"thr{b}", tag="thr")
                nc.vector.tensor_reduce(thr, t3, axis=AX.X, op=ALU.min)
                nc.scalar.dma_start(out=thr_d[b, :], in_=thr)
                nc.scalar.dma_start(
                    out=thrB[b],
                    in_=bass.AP(tensor=thr_d.tensor, offset=thr_d.offset + b,
                                ap=[[0, 128], [1, 1]]))
                for c in range(NCOT):
                    nc.vector.scalar_tensor_tensor(
                        out=mask_cb[:, c * NB + b: c * NB + b + 1],
                        in0=sal_cb[c][:, b:b + 1], scalar=thrB[b][:, 0:1],
                        in1=sal_cb[c][:, b:b + 1], op0=ALU.is_gt, op1=ALU.mult)

            def emit_evict(c, b, banks):
                for n in range(2):
                    slot = b * 2 + n
                    ysl = yv[c][:, b * NSP + n * 512: b * NSP + (n + 1) * 512]
                    nc.scalar.activation(
                        ysl, banks[n], AF.Copy, bias=0.0,
                        scale=mask_cb[:, c * NB + b: c * NB + b + 1],
                        accum_out=s1[c][:, slot:slot + 1])
                    sq = stp.tile([128, 512], BF16, name=f"sq{c}{b}{n}",
                                  tag="sq", bufs=2)
                    nc.vector.scalar_tensor_tensor(
                        out=sq, in0=banks[n],
                        scalar=mask_cb[:, c * NB + b: c * NB + b + 1],
                        in1=ysl, op0=ALU.mult, op1=ALU.mult,
                        accum_out=s2[c][:, slot:slot + 1])

            def emit_ag_front(c):
                r12 = smp.tile([128, 2], F32, name=f"r12_{c}", tag="r12", bufs=4)
                nc.vector.tensor_reduce(r12[:, 0:1], s1[c], axis=AX.X, op=ALU.add)
                nc.vector.tensor_reduce(r12[:, 1:2], s2[c], axis=AX.X, op=ALU.add)
                nc.gpsimd.dma_start(out=cc_in[c, :, :], in_=r12)
                nc.gpsimd.collective_compute(
                    "AllGather", ALU.bypass,
                    replica_groups=[list(range(N_CORES))],
                    ins=[cc_in[c, :, :]], outs=[cc_out[c, :, :, :]])

            def emit_ag_back(c):
                sg = smp.tile([128, 2 * N_CORES], F32, name=f"sg{c}", tag="sg")
                nc.scalar.dma_start(
                    out=sg, in_=bass.AP(tensor=cc_out, offset=c * 2 * 128 * N_CORES,
                                        ap=[[2, 128], [256, N_CORES], [1, 2]]))
                s12 = smp.tile([128, 2], F32, name=f"s12_{c}", tag="s12")
                sgv = bass.AP(tensor=sg.tensor, offset=sg.offset,
                              ap=[sg.ap[0], [1, 2], [2, N_CORES]])
                nc.vector.tensor_reduce(s12, sgv, axis=AX.X, op=ALU.add)
                mu = smp.tile([128, 1], F32, name=f"mu{c}", tag="mu", bufs=1)
                nc.vector.tensor_scalar(mu, s12[:, 0:1], inv_n, None, op0=ALU.mult)
                m2 = smp.tile([128, 1], F32, name=f"m2{c}", tag="m2", bufs=1)
                nc.vector.tensor_scalar(m2, s12[:, 1:2], inv_n, None, op0=ALU.mult)
                var = smp.tile([128, 1], F32, name=f"var{c}", tag="var", bufs=1)
                nc.vector.scalar_tensor_tensor(
                    out=var, in0=mu, scalar=mu[:, :], in1=m2,
                    op0=ALU.mult, op1=ALU.subtract)  # mu*mu - m2 = -var
                sv = smp.tile([128, 1], F32, name=f"sv{c}", tag="sv", bufs=1)
                nc.scalar.activation(sv, var, AF.Sqrt, bias=epst[:, :], scale=-1.0)
                rstd = smp.tile([128, 1], F32, name=f"rstd{c}", tag="rstd", bufs=1)
                nc.vector.reciprocal(rstd, sv)
                nc.vector.tensor_tensor(out=scl[:, c:c + 1],
                                        in0=gam_t[:, c:c + 1], in1=rstd,
                                        op=ALU.mult)
                msc = smp.tile([128, 1], F32, name=f"msc{c}", tag="msc", bufs=1)
                nc.vector.tensor_tensor(out=msc, in0=mu,
                                        in1=scl[:, c:c + 1], op=ALU.mult)
                nc.vector.tensor_tensor(out=shf[:, c:c + 1],
                                        in0=bet_t[:, c:c + 1], in1=msc,
                                        op=ALU.subtract)

            def emit_epi(c):
                for b in range(NB):
                    z = stp.tile([128, NSP], F32, name=f"z{c}{b}", tag="z", bufs=2)
                    for n in range(2):
                        ysl = yv[c][:, b * NSP + n * 512: b * NSP + (n + 1) * 512]
                        zn = z[:, n * 512:(n + 1) * 512]
                        if c == NCOT - 1 and n == 1:
                            nc.vector.tensor_scalar(zn, ysl, scl[:, c:c + 1],
                                                    shf[:, c:c + 1],
                                                    op0=ALU.mult, op1=ALU.add)
                            nc.vector.scalar_tensor_tensor(
                                out=zn, in0=zn, scalar=float(NEG_SLOPE),
                                in1=zn, op0=ALU.mult, op1=ALU.max)
                        else:
                            nc.scalar.activation(
                                zn, ysl, AF.Prelu, bias=shf[:, c:c + 1],
                                scale=scl[:, c:c + 1], alpha=float(NEG_SLOPE))
                    nc.sync.dma_start(
                        out=out[b, c * 128:(c + 1) * 128, :, :].rearrange(
                            "p h w -> p (h w)"),
                        in_=z)

            # ================= emission sequence =================
            emit_wch_dma(0)
            emit_tern(0, 0, nc.vector)
            emit_tern(0, 1, nc.vector)
            emit_T(0, range(8))
            emit_x_block(0, 0)
            emit_wch_dma(1)

            # saliency weights + per-channel vectors
            swn = []
            for c in range(NCOT):
                sw = smp.tile([128, CIN], F32, name=f"swn{c}", tag="salw")
                nc.sync.dma_start(out=sw, in_=salw[c * 128:(c + 1) * 128, :])
                swn.append(sw)

            def col128(dram_vec, nm):  # [512] dram -> [128,4] sbuf
                t_ = pp.tile([128, NCOT], F32, name=nm)
                ap = bass.AP(tensor=dram_vec, offset=0, ap=[[1, 128], [128, NCOT]])
                nc.sync.dma_start(out=t_, in_=ap)
                return t_

            salb_t = col128(salb, "salb_t")
            gam_t = col128(gam, "gam_t")
            bet_t = col128(bet, "bet_t")

            for c in range(NCOT):
                for t in range(NT):
                    pbt = psp.tile([128, 512], F32, name=f"ptw{c}{t}", tag="wbank", bufs=2)
                    nc.tensor.transpose(pbt[:, 0:128],
                                        swn[c][:, t * 128:(t + 1) * 128], identF)
                    nc.scalar.copy(salwT[t][:, c * 128:(c + 1) * 128],
                                   pbt[:, 0:128])

            emit_x_block(0, 1)
            emit_tern(1, 0, nc.vector)
            emit_tern(1, 1, nc.vector)

            seq_banks = {}
            seq_banks[(0, 0)] = emit_unit(0, 0)
            topk_sample(0)
            emit_evict(0, 0, seq_banks.pop((0, 0)))

            emit_wch_dma(2)
            emit_x_block(1, 0)
            emit_tern(2, 0, nc.vector)
            emit_tern(2, 1, nc.vector)
            emit_T(1, range(8))

            seq_banks[(1, 0)] = emit_unit(1, 0)
            emit_evict(1, 0, seq_banks.pop((1, 0)))

            emit_wch_dma(3)
            emit_x_block(1, 1)
            topk_sample(1)
            emit_tern(3, 0, nc.vector)
            emit_tern(3, 1, nc.vector)
            emit_T(2, range(8))

            seq_banks[(2, 0)] = emit_unit(2, 0)
            emit_evict(2, 0, seq_banks.pop((2, 0)))

            emit_x_block(2, 0)
            emit_T(3, range(8))

            seq_banks[(3, 0)] = emit_unit(3, 0)
            emit_evict(3, 0, seq_banks.pop((3, 0)))

            emit_x_block(2, 1)
            topk_sample(2)

            banks = emit_unit(0, 1)
            emit_evict(0, 1, banks)

            emit_x_block(3, 0)

            banks = emit_unit(0, 2)
            emit_evict(0, 2, banks)

            emit_x_block(3, 1)
            topk_sample(3)

            banks = emit_unit(1, 1)
            emit_evict(1, 1, banks)

            banks = emit_unit(0, 3)
            emit_evict(0, 3, banks)
            emit_ag_front(0)

            banks = emit_unit(1, 2)
            emit_evict(1, 2, banks)

            banks = emit_unit(1, 3)
            emit_evict(1, 3, banks)
            emit_ag_front(1)

            for (c, b) in [(2, 1), (2, 2), (2, 3)]:
                banks = emit_unit(c, b)
                emit_evict(c, b, banks)
            emit_ag_front(2)

            for b in (1, 2, 3):
                banks = emit_unit(3, b)
                emit_evict(3, b, banks)
            emit_ag_front(3)

            # finale: stats finalize + epilogue per cot (emitted last so the
            # sg waits never block conv-phase work on any queue)
            for c in range(NCOT):
                emit_ag_back(c)
                emit_epi(c)

            if debug:
                nc.sync.dma_start(out=dbg_mask[:, :], in_=mask_cb)
                for t in range(NT):
                    nc.sync.dma_start(out=bass.AP(tensor=dbg_sub, offset=t * NB,
                                                  ap=[[NT * NB, 128], [1, NB]]),
                                      in_=subT[t])
                for c in range(NCOT):
                    nc.sync.dma_start(out=bass.AP(tensor=dbg_sal, offset=c * NB,
                                                  ap=[[NCOT * NB, 128], [1, NB]]),
                                      in_=sal_cb[c])
                nc.sync.dma_start(out=dbg_thr[:],
                                  in_=bass.AP(tensor=thr_d.tensor,
                                              offset=thr_d.offset,
                                              ap=[[1, NB]]))
                nc.sync.dma_start(out=dbg_wq[:, :], in_=wq[3])
                nc.sync.dma_start(out=dbg_x8[:, :, :, :, :], in_=x8[0])
                nc.sync.dma_start(out=dbg_scl[:, :], in_=scl)
                nc.sync.dma_start(out=dbg_shf[:, :], in_=shf)

    _split_waits(nc)
    return nc


_CACHE = {}


def kernel(x, weight, pos, neg, sal_w, sal_b, gamma, beta):
    x = np.ascontiguousarray(np.asarray(x, dtype=np.float32))
    weight = np.ascontiguousarray(np.asarray(weight, dtype=np.float32))
    sal_w = np.ascontiguousarray(np.asarray(sal_w, dtype=np.float32))
    sal_b = np.ascontiguousarray(np.asarray(sal_b, dtype=np.float32))
    gamma = np.ascontiguousarray(np.asarray(gamma, dtype=np.float32))
    beta = np.ascontiguousarray(np.asarray(beta, dtype=np.float32))
    pos_f = np.float32(np.asarray(pos).reshape(()))
    neg_f = np.float32(np.asarray(neg).reshape(()))

    r8 = _fp8_round(float(np.float32(neg_f / pos_f)))
    eps_imm = float(np.float32(BN_EPS) / (pos_f * pos_f))
    t_imm = float(np.float32(THRESH_FACTOR) * np.float32(np.abs(weight).max()))

    import os
    debug = os.environ.get("KERNEL_DEBUG", "0") == "1"
    key = (r8, eps_imm, t_imm, debug)
    if key not in _CACHE:
        _CACHE[key] = build_kernel(r8, eps_imm, t_imm, debug)
    nc = _CACHE[key]

    in_maps = []
    for c in range(N_CORES):
        in_maps.append({
            "xs": x[c * NB:(c + 1) * NB],
            "wt": weight,
            "salw": sal_w,
            "salb": sal_b,
            "gam": gamma,
            "bet": beta,
        })
    res = run_bass_kernel_spmd(nc, in_maps, core_ids=list(range(N_CORES)))
    if debug:
        kernel.dbg = res.results
    out = np.concatenate([res.results[c]["out"] for c in range(N_CORES)], axis=0)
    return out
